# revision 1
# baseline (speedup 1.0000x reference)
"""Trainium2 Bass kernel for differentiable belief propagation (HMM forward-backward).

Full inputs: unary_logits (16, 4096, 128) f32, log_trans (128, 128) f32.
Output: log-marginals log_softmax(alpha+beta) of shape (16, 4096, 128) f32.

Strategy: data-parallel over batch (2 batch elements per core, 8 cores).
Per core the recursion runs in exp space:
    P = row-softmax(log_trans)                 (C x C stochastic matrix)
    eu_t = exp(u_t - 1/2)                      (drift-centered observation)
    f_t = eu_t * (P^T f_{t-1}),  f_0 = eu_0    (forward, column form)
    h_t = eu_t * (P h_{t+1}),    h_{T-1} = eu_{T-1}   (backward, h = eu*g)
    out_t = log(w_t / sum_j w_t),  w_t = f_t * h_t / eu_t
Per-(b,t) positive scale factors cancel in the normalized log output.

Key acceleration: P is strictly positive, so the recursion contracts in the
projective metric extremely fast (~1e-11 in 16 steps for softmax-of-randn).
The T axis is therefore split into NCH chunks that run IN PARALLEL, each
seeded from an arbitrary start k=HALO steps early; after the burn-in the
chunk states match the sequential scan to far below f32 precision. Each
scan round is one wide matmul (all chunks, both batch elements) plus one
DVE multiply, so the whole scan is ~(HALO + T/NCH) rounds instead of T.

States live with C=128 on partitions; chunk/batch columns are strided
slices of persistent [C, BLOC, T] SBUF arrays.
"""

import numpy as np
from contextlib import ExitStack

import concourse.bass as bass
import concourse.bacc as bacc
import concourse.mybir as mybir
from concourse import tile, masks
from concourse.bass_utils import run_bass_kernel_spmd

F32 = mybir.dt.float32
B, T, C = 16, 4096, 128
NCORES = 8
BLOC = B // NCORES  # batch elements per core
NCH = 256           # time chunks run in parallel
HALO = 8           # burn-in steps per chunk
FP32R = False       # run scan matmuls in float32r (single-pass PE mode)

_ALU = mybir.AluOpType
_ACT = mybir.ActivationFunctionType
_AX = mybir.AxisListType


def _build_program(t_len: int = T, bloc: int = BLOC, nch: int = NCH,
                   halo: int = HALO, reps: int = 1):
    nc = bacc.Bacc(
        "TRN2",
        target_bir_lowering=False,
        debug=False,
        num_devices=NCORES,
    )
    u = nc.dram_tensor("u", (bloc, t_len, C), F32, kind="ExternalInput").ap()
    lt = nc.dram_tensor("lt", (C, C), F32, kind="ExternalInput").ap()
    out = nc.dram_tensor("out", (bloc, t_len, C), F32, kind="ExternalOutput").ap()

    with tile.TileContext(nc) as tc:
        for r in range(reps):
            with ExitStack() as ctx:
                _body(ctx, tc, nc, u, lt, out, t_len, bloc, nch, halo, rep=r)
    nc.compile()
    return nc


def _body(ctx, tc, nc, u, lt, out, t_len, bloc, nch, halo, rep=0,
          parts=("load", "scan", "combine")):
    L = t_len // nch
    assert L * nch == t_len and halo <= L

    cpool = ctx.enter_context(tc.tile_pool(name=f"const{rep}", bufs=1))
    bigpool = ctx.enter_context(tc.tile_pool(name=f"big{rep}", bufs=1))
    stpool = ctx.enter_context(tc.tile_pool(name=f"stage{rep}", bufs=4))
    smpool = ctx.enter_context(tc.tile_pool(name=f"small{rep}", bufs=4))
    scrpool = ctx.enter_context(tc.tile_pool(name=f"scr{rep}", bufs=2))
    ptr = ctx.enter_context(tc.tile_pool(name=f"ps_tr{rep}", bufs=2, space="PSUM"))
    pmm = ctx.enter_context(tc.tile_pool(name=f"ps_mm{rep}", bufs=3, space="PSUM"))

    ident = cpool.tile([C, C], F32)
    masks.make_identity(nc, ident[:])
    neg_half = cpool.tile([C, 1], F32)
    nc.vector.memset(neg_half[:], -0.5)
    zero_col = cpool.tile([C, 1], F32)
    nc.vector.memset(zero_col[:], 0.0)

    # ---- P = row-softmax(lt), PT = P^T ----
    lt_sb = cpool.tile([C, C], F32)
    nc.sync.dma_start(out=lt_sb[:], in_=lt)
    maxv = cpool.tile([C, 1], F32)
    nc.vector.tensor_reduce(maxv[:], lt_sb[:], axis=_AX.X, op=_ALU.max)
    negmax = cpool.tile([C, 1], F32)
    nc.vector.tensor_scalar_mul(negmax[:], maxv[:], -1.0)
    pe_un = cpool.tile([C, C], F32)
    nc.scalar.activation(pe_un[:], lt_sb[:], _ACT.Exp, bias=negmax[:])
    ssum = cpool.tile([C, 1], F32)
    nc.vector.tensor_reduce(ssum[:], pe_un[:], axis=_AX.X, op=_ALU.add)
    rsum = cpool.tile([C, 1], F32)
    nc.vector.reciprocal(rsum[:], ssum[:])
    P_sb = cpool.tile([C, C], F32)
    nc.vector.tensor_scalar_mul(P_sb[:], pe_un[:], rsum[:])
    pt_ps = ptr.tile([C, C], F32, tag="trg")
    nc.tensor.transpose(pt_ps[:], P_sb[:], ident[:])
    PT_sb = cpool.tile([C, C], F32)
    nc.scalar.copy(PT_sb[:], pt_ps[:])
    BF16 = mybir.dt.bfloat16
    P_bf = cpool.tile([C, C], BF16)
    nc.vector.tensor_copy(P_bf[:], P_sb[:])
    PT_bf = cpool.tile([C, C], BF16)
    nc.vector.tensor_copy(PT_bf[:], pt_ps[:])

    def mm(ps_ap, lhsT_ap, rhs_ap):
        if FP32R:
            lhsT_ap = lhsT_ap.bitcast(mybir.dt.float32r)
            rhs_ap = rhs_ap.bitcast(mybir.dt.float32r)
        nc.tensor.matmul(ps_ap, lhsT=lhsT_ap, rhs=rhs_ap)

    # ---- persistent arrays: cols indexed [b, t], states on partitions ----
    euT = bigpool.tile([C, bloc, t_len], F32)
    euTi = bigpool.tile([C, bloc, t_len], F32)   # 1/eu
    Farr = bigpool.tile([C, bloc, t_len], F32)
    Harr = bigpool.tile([C, bloc, t_len], F32)

    # ---- phase 0: transpose u and exponentiate (and reciprocal table) ----
    GRP = min(4, t_len // C)   # (C,128) tiles per PSUM bank group
    DGRP = min(8, t_len // C)  # (C,128) tiles per input DMA
    dma_alt = 0
    for b in range(bloc):
        for d0 in range(0, t_len, C * DGRP):
            # one batched DMA brings DGRP (C,C) tiles; split across the two
            # HWDGE engines (SP / ACT) to halve queue occupancy
            stage = stpool.tile([C, DGRP, C], F32, tag="ustage")
            eng = nc.sync if dma_alt % 2 == 0 else nc.scalar
            dma_alt += 1
            eng.dma_start(
                out=stage[:],
                in_=u[b, d0 : d0 + C * DGRP, :].rearrange(
                    "(blk p) j -> p blk j", p=C
                ),
            )
            for s0 in range(0, DGRP, GRP):
                g0 = d0 + s0 * C
                trg = ptr.tile([C, C * GRP], F32, tag="trg")
                for i in range(GRP):
                    nc.tensor.transpose(
                        trg[:, i * C : (i + 1) * C], stage[:, s0 + i, :],
                        ident[:],
                    )
                nc.scalar.activation(
                    euT[:, b, g0 : g0 + C * GRP], trg[:], _ACT.Exp,
                    bias=neg_half[:],
                )
                nc.vector.reciprocal(
                    euTi[:, b, g0 : g0 + C * GRP], euT[:, b, g0 : g0 + C * GRP]
                )

    if "scan" not in parts:
        # still produce the output tensor so the program shape is unchanged
        o_sb = stpool.tile([C, C], F32, tag="o")
        nc.vector.tensor_copy(o_sb[:], euT[:, 0, 0:C])
        nc.sync.dma_start(out=out[0, 0:C, :], in_=o_sb[:])
        return

    # ---- chunk-parallel scans (fwd/bwd rounds emitted interleaved) ----
    # forward: chunk c owns t in [cL, (c+1)L); burn-in from t = cL - halo.
    # backward: burn-in from t = (c+1)L - 1 + halo; h handed off at chunk top.
    nc.vector.tensor_copy(Farr[:, :, 0], euT[:, :, 0])
    nc.vector.tensor_copy(Harr[:, :, t_len - 1], euT[:, :, t_len - 1])
    BF_ROUNDS = max(0, halo - 3)  # early burn-in in bf16; error contracts away
    fst = bst = None
    for i in range(1, halo):
        use_bf = i <= BF_ROUNDS
        # fwd: state(c, i) = f-approx at t = cL - halo + i, chunks 1..nch-1
        s0 = L - halo + i
        stop = s0 + (nch - 1) * L
        if fst is None:
            seed = scrpool.tile([C, bloc, nch - 1], BF16, tag="fseed")
            nc.vector.tensor_copy(seed[:], euT[:, :, s0 - 1 : stop - 1 : L])
            rhs = seed[:]
        else:
            rhs = fst[:]
        ps = pmm.tile([C, bloc, nch - 1], F32, tag="psf")
        mm(ps[:], P_bf[:] if use_bf else P_sb[:], rhs)
        fst = scrpool.tile(
            [C, bloc, nch - 1], BF16 if i < BF_ROUNDS else F32, tag="fscr"
        )
        nc.vector.tensor_tensor(
            fst[:], ps[:], euT[:, :, s0:stop:L], op=_ALU.mult
        )
        # bwd: state(c, i) = h-approx at t = (c+1)L - 1 + halo - i, c 0..nch-2
        s0 = L - 1 + halo - i
        stop = s0 + (nch - 2) * L + 1
        if bst is None:
            seed = scrpool.tile([C, bloc, nch - 1], BF16, tag="bseed")
            nc.vector.tensor_copy(seed[:], euT[:, :, s0 + 1 : stop + 1 : L])
            rhs = seed[:]
        else:
            rhs = bst[:]
        ps = pmm.tile([C, bloc, nch - 1], F32, tag="psb")
        mm(ps[:], PT_bf[:] if use_bf else PT_sb[:], rhs)
        bst = scrpool.tile(
            [C, bloc, nch - 1], BF16 if i < BF_ROUNDS else F32, tag="bscr"
        )
        nc.vector.tensor_tensor(
            bst[:], ps[:], euT[:, :, s0:stop:L], op=_ALU.mult
        )
    # round j=0: fwd chunks 1.. compute t = cL; bwd chunks ..nch-2, t=(c+1)L-1
    ps = pmm.tile([C, bloc, nch - 1], F32, tag="psf")
    mm(ps[:], P_sb[:], fst[:])
    nc.vector.tensor_tensor(
        Farr[:, :, L : L + (nch - 1) * L : L], ps[:],
        euT[:, :, L : L + (nch - 1) * L : L], op=_ALU.mult,
    )
    ps = pmm.tile([C, bloc, nch - 1], F32, tag="psb")
    mm(ps[:], PT_sb[:], bst[:])
    nc.vector.tensor_tensor(
        Harr[:, :, L - 1 : L - 1 + (nch - 1) * L : L], ps[:],
        euT[:, :, L - 1 : L - 1 + (nch - 1) * L : L], op=_ALU.mult,
    )
    # rounds j=1..L-1: all chunks both directions
    for j in range(1, L):
        ps = pmm.tile([C, bloc, nch], F32, tag="psf")
        mm(ps[:], P_sb[:], Farr[:, :, j - 1 :: L])
        nc.vector.tensor_tensor(
            Farr[:, :, j::L], ps[:], euT[:, :, j::L], op=_ALU.mult
        )
        ps = pmm.tile([C, bloc, nch], F32, tag="psb")
        mm(ps[:], PT_sb[:], Harr[:, :, L - j :: L])
        nc.vector.tensor_tensor(
            Harr[:, :, L - 1 - j :: L], ps[:], euT[:, :, L - 1 - j :: L],
            op=_ALU.mult,
        )

    # ---- combine: out[b, t, :] = log(w) - log(sum_j w), w = F * H / eu ----
    for b in range(bloc):
        for g0 in range(0, t_len, C * GRP):
            w_sb = stpool.tile([C, C * GRP], F32, tag="w")
            nc.vector.tensor_tensor(
                w_sb[:], Farr[:, b, g0 : g0 + C * GRP],
                Harr[:, b, g0 : g0 + C * GRP], op=_ALU.mult,
            )
            nc.gpsimd.tensor_tensor(
                w_sb[:], w_sb[:], euTi[:, b, g0 : g0 + C * GRP],
                op=_ALU.mult,
            )
            wT = ptr.tile([C, C * GRP], F32, tag="trg")
            for i in range(GRP):
                nc.tensor.transpose(
                    wT[:, i * C : (i + 1) * C],
                    w_sb[:, i * C : (i + 1) * C], ident[:],
                )
            scol = smpool.tile([C, GRP], F32, tag="scol")
            nc.vector.tensor_reduce(
                scol[:],
                wT[:].rearrange("p (g c) -> p g c", g=GRP),
                axis=_AX.X, op=_ALU.add,
            )
            lnS = smpool.tile([C, GRP], F32, tag="lnS")
            nc.scalar.activation(lnS[:], scol[:], _ACT.Ln, bias=zero_col[:])
            lnw = stpool.tile([C, GRP, C], F32, tag="lnw")
            nc.scalar.activation(
                lnw[:].rearrange("p g c -> p (g c)"), wT[:], _ACT.Ln,
                bias=zero_col[:],
            )
            og = stpool.tile([C, GRP, C], F32, tag="og")
            sub_eng = nc.vector if dma_alt % 2 == 0 else nc.gpsimd
            sub_eng.tensor_tensor(
                og[:], lnw[:], lnS[:].broadcast_to((C, GRP, C)),
                op=_ALU.subtract,
            )
            eng = nc.sync if dma_alt % 2 == 0 else nc.scalar
            dma_alt += 1
            eng.dma_start(
                out=out[b, g0 : g0 + C * GRP, :].rearrange(
                    "(blk p) j -> p blk j", p=C
                ),
                in_=og[:],
            )


_cached_nc = {}


def _get_program(t_len=T, bloc=BLOC):
    key = (t_len, bloc)
    if key not in _cached_nc:
        _cached_nc[key] = _build_program(t_len, bloc)
    return _cached_nc[key]


def kernel(unary_logits: np.ndarray, log_trans: np.ndarray) -> np.ndarray:
    u = np.ascontiguousarray(unary_logits, dtype=np.float32)
    lt = np.ascontiguousarray(log_trans, dtype=np.float32)
    b_all, t_len, c = u.shape
    bloc = b_all // NCORES
    nc = _get_program(t_len, bloc)
    in_maps = [
        {"u": u[i * bloc : (i + 1) * bloc], "lt": lt} for i in range(NCORES)
    ]
    res = run_bass_kernel_spmd(nc, in_maps, list(range(NCORES)))
    outs = [res.results[i]["out"] for i in range(NCORES)]
    return np.concatenate(outs, axis=0)



# revision 17
# speedup vs baseline: 1.3814x; 1.3814x over previous
"""Trainium2 Bass kernel for differentiable belief propagation (HMM forward-backward).

Full inputs: unary_logits (16, 4096, 128) f32, log_trans (128, 128) f32.
Output: log-marginals log_softmax(alpha+beta) of shape (16, 4096, 128) f32.

Strategy: data-parallel over batch (2 batch elements per core, 8 cores).
Per core the recursion runs in exp space with an fp16 data path:
    P = row-softmax(log_trans)                 (C x C stochastic matrix)
    eu_t = exp(u_t - 1/2)                      (drift-centered observation)
    f_t = eu_t * (P^T f_{t-1}),  f_0 = eu_0    (forward)
    G_t = P h_{t+1},  h_t = eu_t * G_t,  h_{T-1} = eu_{T-1}, G_{T-1} := 1
    w_t = f_t * G_t   (per-(b,t) positive scales cancel after normalization)
    out_t = log(w_t / S_chunk(t))
P is strictly positive so the recursion contracts projectively (~0.2/step);
the T axis splits into NCH chunks of L steps scanned in parallel, each
seeded HALO steps early.  All arrays are SET-MAJOR [C, set, b, chunk]
(t = chunk*L + set) so every scan/combine slice is contiguous.
Forward round j completes set j; backward round j completes G-set L-1-j.
From round L/2 the combine (w-mult, transpose to chunk-major, Ln, DMA out)
of the finished pair (j, L-1-j) is interleaved into the scan; G of
late-consumed sets is read straight out of PSUM, early-consumed sets are
copied once.  sum_j w_t[j] is t-invariant within a chunk (HMM evidence
invariant), so the normalizer is one reduce of one transposed set, and in
chunk-major layout it is a per-partition activation scale: out =
Ln(wT * (1/S)).  Validated error ~1e-3 vs the 2e-2 gate.
"""

import numpy as np
from contextlib import ExitStack

import concourse.bass as bass
import concourse.bacc as bacc
import concourse.mybir as mybir
from concourse import tile, masks
from concourse.bass_utils import run_bass_kernel_spmd

F32 = mybir.dt.float32
F16 = mybir.dt.float16
B, T, C = 16, 4096, 128
NCORES = 8
BLOC = B // NCORES  # batch elements per core
NCH = 256           # time chunks scanned in parallel
HALO = 4            # burn-in steps per chunk

_ALU = mybir.AluOpType
_ACT = mybir.ActivationFunctionType
_AX = mybir.AxisListType


def _build_program(t_len: int = T, bloc: int = BLOC, nch: int = NCH,
                   halo: int = HALO, reps: int = 1):
    nc = bacc.Bacc(
        "TRN2",
        target_bir_lowering=False,
        debug=False,
        num_devices=NCORES,
    )
    u = nc.dram_tensor("u", (bloc, t_len, C), F32, kind="ExternalInput").ap()
    lt = nc.dram_tensor("lt", (C, C), F32, kind="ExternalInput").ap()
    out = nc.dram_tensor("out", (bloc, t_len, C), F32, kind="ExternalOutput").ap()

    with tile.TileContext(nc) as tc:
        for r in range(reps):
            with ExitStack() as ctx:
                _body(ctx, tc, nc, u, lt, out, t_len, bloc, nch, halo, rep=r)
    nc.compile()
    return nc


def _body(ctx, tc, nc, u, lt, out, t_len, bloc, nch, halo, rep=0):
    L = t_len // nch
    assert L * nch == t_len and halo <= L and L % 2 == 0
    NW = bloc * nch              # scan matmul width
    NB = nch // C                # 128-chunk blocks per batch elem
    Q = bloc * NB                # transposed [C,C] quadrants per set
    half = L // 2                # first combined round; S reference set

    cpool = ctx.enter_context(tc.tile_pool(name=f"const{rep}", bufs=1))
    bigpool = ctx.enter_context(tc.tile_pool(name=f"big{rep}", bufs=1))
    stpool = ctx.enter_context(tc.tile_pool(name=f"stage{rep}", bufs=6))
    scrpool = ctx.enter_context(tc.tile_pool(name=f"scr{rep}", bufs=2))
    smpool = ctx.enter_context(tc.tile_pool(name=f"small{rep}", bufs=2))

    ident = cpool.tile([C, C], F32)
    masks.make_identity(nc, ident[:])
    ident16 = cpool.tile([C, C], F16)
    nc.vector.tensor_copy(ident16[:], ident[:])
    neg_half = cpool.tile([C, 1], F32)
    nc.vector.memset(neg_half[:], -0.5)
    ones16 = cpool.tile([C, C], F16)
    nc.vector.memset(ones16[:], 1.0)

    # ---- persistent set-major arrays: X[:, s, b, c] holds t = c*L + s ----
    euT = bigpool.tile([C, L, bloc, nch], F32)
    Farr = bigpool.tile([C, L, bloc, nch], F16)
    # G sets half+1..L-1 are produced L-1-2s rounds before use; keep them.
    Garr = bigpool.tile([C, half - 1, bloc, nch], F16)

    # ---- P = row-softmax(lt) in f32, cast to fp16, and its transpose ----
    with tc.tile_pool(name=f"pprep{rep}", bufs=1) as ppool, \
         tc.tile_pool(name=f"ps_pp{rep}", bufs=1, space="PSUM") as pps:
        lt_sb = ppool.tile([C, C], F32)
        nc.sync.dma_start(out=lt_sb[:], in_=lt)
        maxv = ppool.tile([C, 1], F32)
        nc.vector.tensor_reduce(maxv[:], lt_sb[:], axis=_AX.X, op=_ALU.max)
        negmax = ppool.tile([C, 1], F32)
        nc.vector.tensor_scalar_mul(negmax[:], maxv[:], -1.0)
        pe_un = ppool.tile([C, C], F32)
        nc.scalar.activation(pe_un[:], lt_sb[:], _ACT.Exp, bias=negmax[:])
        ssum = ppool.tile([C, 1], F32)
        nc.vector.tensor_reduce(ssum[:], pe_un[:], axis=_AX.X, op=_ALU.add)
        rsum = ppool.tile([C, 1], F32)
        nc.vector.reciprocal(rsum[:], ssum[:])
        P16 = cpool.tile([C, C], F16)
        nc.vector.tensor_scalar_mul(P16[:], pe_un[:], rsum[:])
        pt_ps = pps.tile([C, C], F16, tag="ptr")
        nc.tensor.transpose(pt_ps[:], P16[:], ident16[:])
        PT16 = cpool.tile([C, C], F16)
        nc.scalar.copy(PT16[:], pt_ps[:])

    # ---- phase 0: DMA u, exp in t-major, transpose, scatter set-major ----
    DGRP = 8                    # (C,128) t-blocks per input DMA / PSUM group
    dma_alt = 0
    with tc.tile_pool(name=f"ps_tr{rep}", bufs=2, space="PSUM") as ptr0:
        for b in range(bloc):
            for d0 in range(0, t_len, C * DGRP):
                stage = stpool.tile([C, DGRP, C], F32, tag="ustage")
                eng = nc.sync if dma_alt % 2 == 0 else nc.scalar
                dma_alt += 1
                eng.dma_start(
                    out=stage[:],
                    in_=u[b, d0 : d0 + C * DGRP, :].rearrange(
                        "(blk p) j -> p blk j", p=C
                    ),
                )
                eu_st = stpool.tile([C, DGRP, C], F32, tag="eust")
                nc.scalar.activation(
                    eu_st[:].rearrange("p blk j -> p (blk j)"),
                    stage[:].rearrange("p blk j -> p (blk j)"),
                    _ACT.Exp, bias=neg_half[:],
                )
                trg = ptr0.tile([C, DGRP, C], F32, tag="trg")
                for i in range(DGRP):
                    nc.tensor.transpose(
                        trg[:, i, :], eu_st[:, i, :], ident[:]
                    )
                # block bl covers chunks c0+8*bl..+7; in-block t = 16*ch + s
                c0 = d0 // L
                dst = euT[:, :, b, c0 : c0 + DGRP * C // L].rearrange(
                    "p s (bl ch) -> p s bl ch", bl=DGRP)
                srcv = trg[:].rearrange("p bl (ch s) -> p s bl ch", s=L)
                if (d0 // (C * DGRP)) % 2 == 0:
                    nc.vector.tensor_copy(dst, srcv)
                else:
                    nc.scalar.copy(dst, srcv)

    # ---- interleaved scan + middle-out combine ----
    nc.vector.tensor_copy(Farr[:, 0, :, 0], euT[:, 0, :, 0])
    with tc.tile_pool(name=f"ps_mm{rep}", bufs=2, space="PSUM") as pmm, \
         tc.tile_pool(name=f"ps_c{rep}", bufs=3, space="PSUM") as ptrc, \
         tc.tile_pool(name=f"ps_sp{rep}", bufs=1, space="PSUM") as pspr:
        fst = bst = None
        for i in range(1, halo):
            # fwd burn-in: state col c-1 ~ f at t = cL-halo+i (chunks 1..)
            s = L - halo + i
            if fst is None:
                seed = scrpool.tile([C, bloc, nch - 1], F16, tag="fseed")
                nc.vector.tensor_copy(seed[:], euT[:, s - 1, :, 0 : nch - 1])
                rhs = seed[:]
            else:
                rhs = fst[:]
            ps = pmm.tile([C, bloc, nch - 1], F32, tag="psf")
            nc.tensor.matmul(ps[:], lhsT=P16[:], rhs=rhs)
            fst = scrpool.tile([C, bloc, nch - 1], F16, tag="fscr")
            nc.vector.tensor_tensor(
                fst[:], ps[:], euT[:, s, :, 0 : nch - 1], op=_ALU.mult)
            # bwd burn-in: state col c ~ h at t = (c+1)L-1+halo-i (c=0..n-2)
            s = halo - 1 - i
            if bst is None:
                seed = scrpool.tile([C, bloc, nch - 1], F16, tag="bseed")
                nc.vector.tensor_copy(seed[:], euT[:, s + 1, :, 1:nch])
                rhs = seed[:]
            else:
                rhs = bst[:]
            ps = pmm.tile([C, bloc, nch - 1], F32, tag="psb")
            nc.tensor.matmul(ps[:], lhsT=PT16[:], rhs=rhs)
            bst = scrpool.tile([C, bloc, nch - 1], F16, tag="bscr")
            nc.vector.tensor_tensor(
                bst[:], ps[:], euT[:, s, :, 1:nch], op=_ALU.mult)

        # round 0
        ps = pmm.tile([C, bloc, nch - 1], F32, tag="psf")
        nc.tensor.matmul(ps[:], lhsT=P16[:], rhs=fst[:])
        nc.vector.tensor_tensor(
            Farr[:, 0, :, 1:nch], ps[:], euT[:, 0, :, 1:nch], op=_ALU.mult)
        ps = pmm.tile([C, bloc, nch - 1], F32, tag="psb")
        nc.tensor.matmul(ps[:], lhsT=PT16[:], rhs=bst[:])
        # G set L-1 -> Garr slot (L-1)-(half+1); G_{T-1} := 1
        nc.scalar.copy(Garr[:, half - 2, :, 0 : nch - 1], ps[:])
        nc.vector.memset(Garr[:, half - 2, :, nch - 1], 1.0)
        hsc = [scrpool.tile([C, bloc, nch], F16, tag=f"hsc{k}",
                            name=f"hsc{k}") for k in range(2)]
        nc.vector.tensor_tensor(
            hsc[0][:, :, 0 : nch - 1], ps[:], euT[:, L - 1, :, 0 : nch - 1],
            op=_ALU.mult)
        nc.vector.tensor_copy(hsc[0][:, :, nch - 1], euT[:, L - 1, :, nch - 1])

        # main rounds j=1..L-1; combine pair (j, L-1-j) from round j>=half
        ps_of_set = {}
        rS_col = None
        for j in range(1, L):
            ps = pmm.tile([C, bloc, nch], F32, tag="psf")
            nc.tensor.matmul(ps[:], lhsT=P16[:], rhs=Farr[:, j - 1])
            nc.vector.tensor_tensor(
                Farr[:, j], ps[:], euT[:, j], op=_ALU.mult)
            ps = pmm.tile([C, bloc, nch], F32, tag="psb")
            nc.tensor.matmul(ps[:], lhsT=PT16[:], rhs=hsc[(j - 1) % 2][:])
            sG = L - 1 - j
            ps_of_set[sG] = ps
            if sG > half:
                nc.scalar.copy(Garr[:, sG - half - 1], ps[:])
            if j < L - 1:
                nc.vector.tensor_tensor(
                    hsc[j % 2][:], ps[:], euT[:, sG], op=_ALU.mult)

            if j < half:
                continue
            # ---- combine pair (sa=j from SBUF/held-PSUM, sb=L-1-j) ----
            sa, sb = j, L - 1 - j
            wa = stpool.tile([C, NW], F16, tag="wa")
            aeng = nc.gpsimd if sa > half else nc.vector
            asrc = Garr[:, sa - half - 1] if sa > half else ps_of_set[sa]
            aeng.tensor_tensor(
                wa[:], Farr[:, sa].rearrange("p b c -> p (b c)"),
                asrc[:].rearrange("p b c -> p (b c)"), op=_ALU.mult)
            wb = stpool.tile([C, NW], F16, tag="wb")
            nc.vector.tensor_tensor(
                wb[:], Farr[:, sb].rearrange("p b c -> p (b c)"),
                ps_of_set[sb][:].rearrange("p b c -> p (b c)"), op=_ALU.mult)
            wTr = ptrc.tile([C, 2, Q, C], F16, tag="wtr")
            for q in range(Q):
                nc.tensor.transpose(
                    wTr[:, 0, q, :], wa[:, q * C : (q + 1) * C], ident16[:])
                nc.tensor.transpose(
                    wTr[:, 1, q, :], wb[:, q * C : (q + 1) * C], ident16[:])
            og = stpool.tile([C, 2, Q, C], F32, tag="og")
            if j == half:
                # normalizer: S per chunk from set `half` (chunk-major rows)
                S_col = smpool.tile([C, Q], F32, tag="scol")
                nc.vector.tensor_reduce(
                    S_col[:], wTr[:, 0], axis=_AX.X, op=_ALU.add)
                rS_col = smpool.tile([C, Q], F32, tag="rscol")
                nc.vector.reciprocal(rS_col[:], S_col[:])
            for q in range(Q):
                nc.scalar.activation(
                    og[:, :, q, :], wTr[:, :, q, :], _ACT.Ln,
                    scale=rS_col[:, q : q + 1])
            for si, s in ((0, sa), (1, sb)):
                nc.sync.dma_start(
                    out=out[:, s::L, :].rearrange(
                        "b (m p) j -> p b m j", p=C),
                    in_=og[:, si].rearrange("p (b m) j -> p b m j", b=bloc),
                )


_cached_nc = {}


def _get_program(t_len=T, bloc=BLOC):
    key = (t_len, bloc)
    if key not in _cached_nc:
        _cached_nc[key] = _build_program(t_len, bloc)
    return _cached_nc[key]


def kernel(unary_logits: np.ndarray, log_trans: np.ndarray) -> np.ndarray:
    u = np.ascontiguousarray(unary_logits, dtype=np.float32)
    lt = np.ascontiguousarray(log_trans, dtype=np.float32)
    b_all, t_len, c = u.shape
    bloc = b_all // NCORES
    nc = _get_program(t_len, bloc)
    in_maps = [
        {"u": u[i * bloc : (i + 1) * bloc], "lt": lt} for i in range(NCORES)
    ]
    res = run_bass_kernel_spmd(nc, in_maps, list(range(NCORES)))
    outs = [res.results[i]["out"] for i in range(NCORES)]
    return np.concatenate(outs, axis=0)


# revision 33
# speedup vs baseline: 1.5428x; 1.1169x over previous
"""Trainium2 Bass kernel for differentiable belief propagation (HMM forward-backward).

Full inputs: unary_logits (16, 4096, 128) f32, log_trans (128, 128) f32.
Output: log-marginals log_softmax(alpha+beta) of shape (16, 4096, 128) f32.

Strategy: data-parallel over batch (2 batch elements per core, 8 cores).
Per core the recursion runs in exp space with an fp16 data path:
    P = row-softmax(log_trans)                 (C x C stochastic matrix)
    eu_t = exp(u_t - 1/2)                      (drift-centered observation)
    f_t = eu_t * (P^T f_{t-1}),  f_0 = eu_0    (forward)
    G_t = P h_{t+1},  h_t = eu_t * G_t,  h_{T-1} = eu_{T-1}, G_{T-1} := 1
    w_t = f_t * G_t   (per-(b,t) positive scales cancel after normalization)
    out_t = log(w_t / S_chunk(t))
P is strictly positive so the recursion contracts projectively (~0.2/step);
the T axis splits into NCH chunks of L steps scanned in parallel, each
seeded HALO steps early.  All arrays are SET-MAJOR [C, set, b, chunk]
(t = chunk*L + set) so every scan/combine slice is contiguous.
Forward round j completes set j; backward round j completes G-set L-1-j.
From round L/2 the combine (w-mult, transpose to chunk-major, Ln, DMA out)
of the finished pair (j, L-1-j) is interleaved into the scan; G of
late-consumed sets is read straight out of PSUM, early-consumed sets are
copied once.  sum_j w_t[j] is t-invariant within a chunk (HMM evidence
invariant), so the normalizer is one reduce of one transposed set, and in
chunk-major layout it is a per-partition activation scale: out =
Ln(wT * (1/S)).  Validated error ~1e-3 vs the 2e-2 gate.
"""

import numpy as np
from contextlib import ExitStack

import concourse.bass as bass
import concourse.bacc as bacc
import concourse.mybir as mybir
from concourse import tile, masks
from concourse.bass_utils import run_bass_kernel_spmd

F32 = mybir.dt.float32
F16 = mybir.dt.float16
B, T, C = 16, 4096, 128
NCORES = 8
BLOC = B // NCORES  # batch elements per core
NCH = 256           # time chunks scanned in parallel
HALO = 4            # burn-in steps per chunk

_ALU = mybir.AluOpType
_ACT = mybir.ActivationFunctionType
_AX = mybir.AxisListType


def _build_program(t_len: int = T, bloc: int = BLOC, nch: int = NCH,
                   halo: int = HALO, reps: int = 1):
    nc = bacc.Bacc(
        "TRN2",
        target_bir_lowering=False,
        debug=False,
        num_devices=NCORES,
    )
    u = nc.dram_tensor("u", (bloc, t_len, C), F32, kind="ExternalInput").ap()
    lt = nc.dram_tensor("lt", (C, C), F32, kind="ExternalInput").ap()
    out = nc.dram_tensor("out", (bloc, t_len, C), F32, kind="ExternalOutput").ap()

    with tile.TileContext(nc) as tc:
        for r in range(reps):
            with ExitStack() as ctx:
                _body(ctx, tc, nc, u, lt, out, t_len, bloc, nch, halo, rep=r)
    nc.compile()
    return nc


def _body(ctx, tc, nc, u, lt, out, t_len, bloc, nch, halo, rep=0):
    L = t_len // nch
    assert L * nch == t_len and halo <= L and L % 2 == 0
    NW = bloc * nch              # scan matmul width
    NB = nch // C                # 128-chunk blocks per batch elem
    Q = bloc * NB                # transposed [C,C] quadrants per set
    half = L // 2                # first combined round; S reference set

    cpool = ctx.enter_context(tc.tile_pool(name=f"const{rep}", bufs=1))
    bigpool = ctx.enter_context(tc.tile_pool(name=f"big{rep}", bufs=1))
    stpool = ctx.enter_context(tc.tile_pool(name=f"stage{rep}", bufs=6))
    scrpool = ctx.enter_context(tc.tile_pool(name=f"scr{rep}", bufs=2))
    smpool = ctx.enter_context(tc.tile_pool(name=f"small{rep}", bufs=2))

    ident = cpool.tile([C, C], F32)
    masks.make_identity(nc, ident[:])
    ident16 = cpool.tile([C, C], F16)
    nc.vector.tensor_copy(ident16[:], ident[:])
    neg_half = cpool.tile([C, 1], F32)
    nc.vector.memset(neg_half[:], -0.5)
    ones16 = cpool.tile([C, C], F16)
    nc.vector.memset(ones16[:], 1.0)


    # ---- persistent set-major arrays: X[:, s, b, c] holds t = c*L + s ----
    euT = bigpool.tile([C, L, bloc, nch], F32)
    Farr = bigpool.tile([C, L, bloc, nch], F16)
    # G sets half+1..L-1 are produced L-1-2s rounds before use; keep them.
    Garr = bigpool.tile([C, half - 1, bloc, nch], F16)

    # ---- P = row-softmax(lt) in f32, cast to fp16, and its transpose ----
    with tc.tile_pool(name=f"pprep{rep}", bufs=1) as ppool, \
         tc.tile_pool(name=f"ps_pp{rep}", bufs=1, space="PSUM") as pps:
        lt_sb = ppool.tile([C, C], F32)
        nc.scalar.dma_start(out=lt_sb[:], in_=lt)
        maxv = ppool.tile([C, 1], F32)
        nc.vector.tensor_reduce(maxv[:], lt_sb[:], axis=_AX.X, op=_ALU.max)
        negmax = ppool.tile([C, 1], F32)
        nc.vector.tensor_scalar_mul(negmax[:], maxv[:], -1.0)
        pe_un = ppool.tile([C, C], F32)
        nc.scalar.activation(pe_un[:], lt_sb[:], _ACT.Exp, bias=negmax[:])
        ssum = ppool.tile([C, 1], F32)
        nc.vector.tensor_reduce(ssum[:], pe_un[:], axis=_AX.X, op=_ALU.add)
        rsum = ppool.tile([C, 1], F32)
        nc.vector.reciprocal(rsum[:], ssum[:])
        P16 = cpool.tile([C, C], F16)
        nc.vector.tensor_scalar_mul(P16[:], pe_un[:], rsum[:])
        pt_ps = pps.tile([C, C], F16, tag="ptr")
        nc.tensor.transpose(pt_ps[:], P16[:], ident16[:])
        PT16 = cpool.tile([C, C], F16)
        nc.scalar.copy(PT16[:], pt_ps[:])

    # ---- phase 0 + burn-in, emitted per batch element: b=0's burn-in
    # chains dispatch as soon as b=0's euT lands, under b=1's input DMA ----
    DGRP = 8                    # (C,128) t-blocks per input DMA / PSUM group
    dma_alt = 0
    hsc = [scrpool.tile([C, bloc, nch], F16, tag=f"hsc{k}",
                        name=f"hsc{k}") for k in range(2)]
    with tc.tile_pool(name=f"ps_tr{rep}", bufs=2, space="PSUM") as ptr0, \
         tc.tile_pool(name=f"ps_bi{rep}", bufs=1, space="PSUM") as pbi:
        for b in range(bloc):
            for d0 in range(0, t_len, C * DGRP):
                stage = stpool.tile([C, DGRP, C], F32, tag="ustage")
                dma_alt += 1
                nc.sync.dma_start(
                    out=stage[:],
                    in_=u[b, d0 : d0 + C * DGRP, :].rearrange(
                        "(blk p) j -> p blk j", p=C
                    ),
                )
                eu_st = stpool.tile([C, DGRP, C], F32, tag="eust")
                nc.scalar.activation(
                    eu_st[:].rearrange("p blk j -> p (blk j)"),
                    stage[:].rearrange("p blk j -> p (blk j)"),
                    _ACT.Exp, bias=neg_half[:],
                )
                trg = ptr0.tile([C, DGRP, C], F32, tag="trg")
                for i in range(DGRP):
                    nc.tensor.transpose(
                        trg[:, i, :], eu_st[:, i, :], ident[:]
                    )
                # block bl covers chunks c0+8*bl..+7; in-block t = 16*ch + s
                c0 = d0 // L
                dst = euT[:, :, b, c0 : c0 + DGRP * C // L].rearrange(
                    "p s (bl ch) -> p s bl ch", bl=DGRP)
                srcv = trg[:].rearrange("p bl (ch s) -> p s bl ch", s=L)
                if (d0 // (C * DGRP)) % 2 == 0:
                    nc.vector.tensor_copy(dst, srcv)
                else:
                    nc.scalar.copy(dst, srcv)
            nc.vector.tensor_copy(
                Farr[:, 0, b : b + 1, 0:1], euT[:, 0, b : b + 1, 0:1])
            if True:
                fst = bst = None
                for i in range(1, halo):
                    # fwd: state col c-1 ~ f at t = cL-halo+i (chunks 1..)
                    s = L - halo + i
                    if fst is None:
                        seed = scrpool.tile([C, 1, nch - 1], F16,
                                            tag=f"fseed{b}", name=f"fsee{b}")
                        nc.vector.tensor_copy(
                            seed[:], euT[:, s - 1, b : b + 1, 0 : nch - 1])
                        rhs = seed[:]
                    else:
                        rhs = fst[:]
                    ps = pbi.tile([C, 1, nch - 1], F32, tag="pf",
                                  name=f"pf{b}_{i}")
                    nc.tensor.matmul(ps[:], lhsT=P16[:], rhs=rhs)
                    fst = scrpool.tile([C, 1, nch - 1], F16, tag=f"fscr{b}",
                                       name=f"fscr{b}_{i}")
                    nc.vector.tensor_tensor(
                        fst[:], ps[:], euT[:, s, b : b + 1, 0 : nch - 1],
                        op=_ALU.mult)
                    # bwd: state col c ~ h at t = (c+1)L-1+halo-i (c<=n-2)
                    s = halo - 1 - i
                    if bst is None:
                        seed = scrpool.tile([C, 1, nch - 1], F16,
                                            tag=f"bseed{b}", name=f"bsee{b}")
                        nc.vector.tensor_copy(
                            seed[:], euT[:, s + 1, b : b + 1, 1:nch])
                        rhs = seed[:]
                    else:
                        rhs = bst[:]
                    ps = pbi.tile([C, 1, nch - 1], F32, tag="pb",
                                  name=f"pb{b}_{i}")
                    nc.tensor.matmul(ps[:], lhsT=PT16[:], rhs=rhs)
                    bst = scrpool.tile([C, 1, nch - 1], F16, tag=f"bscr{b}",
                                       name=f"bscr{b}_{i}")
                    nc.vector.tensor_tensor(
                        bst[:], ps[:], euT[:, s, b : b + 1, 1:nch],
                        op=_ALU.mult)
                # round 0 for this b
                ps = pbi.tile([C, 1, nch - 1], F32, tag="pf",
                              name=f"pf{b}_r0")
                nc.tensor.matmul(ps[:], lhsT=P16[:], rhs=fst[:])
                nc.vector.tensor_tensor(
                    Farr[:, 0, b : b + 1, 1:nch], ps[:],
                    euT[:, 0, b : b + 1, 1:nch], op=_ALU.mult)
                ps = pbi.tile([C, 1, nch - 1], F32, tag="pb",
                              name=f"pb{b}_r0")
                nc.tensor.matmul(ps[:], lhsT=PT16[:], rhs=bst[:])
                nc.scalar.copy(
                    Garr[:, half - 2, b : b + 1, 0 : nch - 1], ps[:])
                nc.vector.tensor_tensor(
                    hsc[0][:, b : b + 1, 0 : nch - 1], ps[:],
                    euT[:, L - 1, b : b + 1, 0 : nch - 1], op=_ALU.mult)
                nc.vector.tensor_copy(
                    hsc[0][:, b : b + 1, nch - 1 : nch],
                    euT[:, L - 1, b : b + 1, nch - 1 : nch])
    # G_{T-1} := 1
    nc.vector.memset(Garr[:, half - 2, :, nch - 1], 1.0)

    # main rounds j=1..L-1; combine pair (j, L-1-j) from round j>=half
    ps_of_set = {}
    rS_col = None
    with tc.tile_pool(name=f"ps_mm{rep}", bufs=2, space="PSUM") as pmm, \
         tc.tile_pool(name=f"ps_c{rep}", bufs=4, space="PSUM") as ptrc:
        for j in range(1, L):
            ps = pmm.tile([C, bloc, nch], F32, tag="psf")
            nc.tensor.matmul(ps[:], lhsT=P16[:], rhs=Farr[:, j - 1])
            nc.vector.tensor_tensor(
                Farr[:, j], ps[:], euT[:, j], op=_ALU.mult)
            ps = pmm.tile([C, bloc, nch], F32, tag="psb")
            nc.tensor.matmul(ps[:], lhsT=PT16[:], rhs=hsc[(j - 1) % 2][:])
            sG = L - 1 - j
            ps_of_set[sG] = ps
            if sG > half:
                nc.scalar.copy(Garr[:, sG - half - 1], ps[:])
            if j < L - 1:
                nc.vector.tensor_tensor(
                    hsc[j % 2][:], ps[:], euT[:, sG], op=_ALU.mult)

            if j < half:
                continue
            # ---- combine pair (sa=j from SBUF/held-PSUM, sb=L-1-j) ----
            sa, sb = j, L - 1 - j
            wa = stpool.tile([C, NW], F16, tag="wa")
            aeng = nc.gpsimd if sa > half else nc.vector
            asrc = Garr[:, sa - half - 1] if sa > half else ps_of_set[sa]
            aeng.tensor_tensor(
                wa[:], Farr[:, sa].rearrange("p b c -> p (b c)"),
                asrc[:].rearrange("p b c -> p (b c)"), op=_ALU.mult)
            wb = stpool.tile([C, NW], F16, tag="wb")
            nc.vector.tensor_tensor(
                wb[:], Farr[:, sb].rearrange("p b c -> p (b c)"),
                ps_of_set[sb][:].rearrange("p b c -> p (b c)"), op=_ALU.mult)
            wTr = ptrc.tile([C, 2, Q, C], F16, tag="wtr")
            for q in range(Q):
                nc.tensor.transpose(
                    wTr[:, 0, q, :], wa[:, q * C : (q + 1) * C], ident16[:])
                nc.tensor.transpose(
                    wTr[:, 1, q, :], wb[:, q * C : (q + 1) * C], ident16[:])
            og = stpool.tile([C, 2, Q, C], F32, tag="og")
            if j == half:
                # normalizer: S per chunk from set `half` (chunk-major rows)
                S_col = smpool.tile([C, Q], F32, tag="scol")
                nc.vector.tensor_reduce(
                    S_col[:], wTr[:, 0], axis=_AX.X, op=_ALU.add)
                rS_col = smpool.tile([C, Q], F32, tag="rscol")
                nc.vector.reciprocal(rS_col[:], S_col[:])
            for q in range(Q):
                nc.scalar.activation(
                    og[:, :, q, :], wTr[:, :, q, :], _ACT.Ln,
                    scale=rS_col[:, q : q + 1])
            for si, s in ((0, sa), (1, sb)):
                nc.sync.dma_start(
                    out=out[:, s::L, :].rearrange(
                        "b (m p) j -> p b m j", p=C),
                    in_=og[:, si].rearrange("p (b m) j -> p b m j", b=bloc),
                )


_cached_nc = {}


def _get_program(t_len=T, bloc=BLOC):
    key = (t_len, bloc)
    if key not in _cached_nc:
        _cached_nc[key] = _build_program(t_len, bloc)
    return _cached_nc[key]


def kernel(unary_logits: np.ndarray, log_trans: np.ndarray) -> np.ndarray:
    u = np.ascontiguousarray(unary_logits, dtype=np.float32)
    lt = np.ascontiguousarray(log_trans, dtype=np.float32)
    b_all, t_len, c = u.shape
    bloc = b_all // NCORES
    nc = _get_program(t_len, bloc)
    in_maps = [
        {"u": u[i * bloc : (i + 1) * bloc], "lt": lt} for i in range(NCORES)
    ]
    res = run_bass_kernel_spmd(nc, in_maps, list(range(NCORES)))
    outs = [res.results[i]["out"] for i in range(NCORES)]
    return np.concatenate(outs, axis=0)
